# revision 1
# baseline (speedup 1.0000x reference)
"""Trainium2 Bass kernel for nn_AttentionBlock (B=16, C=512, H=W=64, 8 heads).

Channel-attention block: GroupNorm(8 groups) -> 1x1 qkv -> scores over
channel dims (contract spatial N=4096) -> softmax -> att @ v -> 1x1 out
projection -> residual.

Sharding: data-parallel over batch. 16 batches / 8 cores = 2 per core.
No collectives. Each core runs the identical program on its 2 batches.

Layouts on device (per batch):
  x     [C, N] fp32, 4 channel-chunk tiles of [128, 4096]
  h     (groupnorm output) same layout, bf16
  q,k   [N, 2C] orientation (spatial on partitions), bf16, transient tiles
  v     [C, N] bf16, resident
  scores 8 heads of [64, 64] packed into two [128, 128] psum tiles
  hv    [C, N] bf16 via paired-head matmuls
  out   = w_out @ hv + (w_out @ (att @ b_v) + b_out) + x   (residual)

All matmuls bf16 inputs with fp32 psum accumulation; groupnorm stats,
softmax, and the residual path are fp32.
"""

import numpy as np
import ml_dtypes

import concourse.bacc as bacc
import concourse.tile as tile
from concourse import mybir
from concourse.bass_utils import run_bass_kernel_spmd
from concourse.masks import make_identity

BF = mybir.dt.bfloat16
F32 = mybir.dt.float32
AX = mybir.AxisListType
OP = mybir.AluOpType
AF = mybir.ActivationFunctionType

C = 512
NH = 8
D = 64  # head dim
G = 8   # groupnorm groups
CK = C // 128  # 4 channel chunks
EPS = 1e-5
N_CORES = 8

# attT slot coords inside a [128,128] attT tile, per chunk parity.
# chunk ck holds heads (2ck, 2ck+1); tile tt = ck // 2.
# even head lhsT lives at partitions 0:64, odd head at partitions 64:128.
_EVEN_SLOT = {0: (0, 0), 1: (0, 64)}   # ck%2 -> (prow, colstart)
_ODD_SLOT = {0: (64, 64), 1: (64, 0)}
# scores placement: local head l (0..3) -> (prow, colstart) in scores tile
_SCORE_SLOT = {0: (0, 0), 1: (64, 64), 2: (64, 0), 3: (0, 64)}


def build_program(B=2, N=4096, debug=False):
    SP = N // 128   # spatial chunks for qk/scores
    NT = N // 512   # 512-col tiles
    SUB = N // 512  # bn_stats subgroups (free dim <= 512)
    scale = float(1.0 / np.sqrt(D))

    nc = bacc.Bacc("TRN2", target_bir_lowering=False, debug=debug,
                   num_devices=N_CORES)

    x_d = nc.dram_tensor("x", [B, C, N], F32, kind="ExternalInput")
    wqk_d = nc.dram_tensor("wqkT", [C, 2 * C], BF, kind="ExternalInput")
    wv_d = nc.dram_tensor("wvT", [C, C], BF, kind="ExternalInput")
    wo_d = nc.dram_tensor("woT", [C, C], BF, kind="ExternalInput")
    bqk_d = nc.dram_tensor("bqk", [1, 2 * C], BF, kind="ExternalInput")
    bv_d = nc.dram_tensor("bv", [C, 1], BF, kind="ExternalInput")
    bo_d = nc.dram_tensor("bo", [C, 1], F32, kind="ExternalInput")
    gam_d = nc.dram_tensor("gamma", [C, 1], F32, kind="ExternalInput")
    bet_d = nc.dram_tensor("beta", [C, 1], F32, kind="ExternalInput")
    indf_d = nc.dram_tensor("indf", [C, G], F32, kind="ExternalInput")
    indb_d = nc.dram_tensor("indb", [G, C], F32, kind="ExternalInput")
    out_d = nc.dram_tensor("out", [B, C, N], F32, kind="ExternalOutput")

    with tile.TileContext(nc) as tc:
        import contextlib
        ctx = contextlib.ExitStack()
        with ctx:
            persist = ctx.enter_context(tc.tile_pool(name="persist", bufs=1))
            big = ctx.enter_context(tc.tile_pool(name="big", bufs=1))
            mid = ctx.enter_context(tc.tile_pool(name="mid", bufs=3))
            small = ctx.enter_context(tc.tile_pool(name="small", bufs=1))
            ps_qk = ctx.enter_context(
                tc.tile_pool(name="ps_qk", bufs=3, space="PSUM"))
            ps_sc = ctx.enter_context(
                tc.tile_pool(name="ps_sc", bufs=1, space="PSUM"))
            ps_big = ctx.enter_context(
                tc.tile_pool(name="ps_big", bufs=2, space="PSUM"))

            # ---- persistent: weights / constants ----
            wqk = []
            wv = []
            wo = []
            bv_sb = []
            bo_sb = []
            gam = []
            bet = []
            for k in range(CK):
                t = persist.tile([128, 2 * C], BF, tag=f"wqk{k}")
                nc.gpsimd.dma_start(out=t, in_=wqk_d.ap()[k * 128:(k + 1) * 128, :])
                wqk.append(t)
                t = persist.tile([128, C], BF, tag=f"wv{k}")
                nc.gpsimd.dma_start(out=t, in_=wv_d.ap()[k * 128:(k + 1) * 128, :])
                wv.append(t)
                t = persist.tile([128, C], BF, tag=f"wo{k}")
                nc.gpsimd.dma_start(out=t, in_=wo_d.ap()[k * 128:(k + 1) * 128, :])
                wo.append(t)
                t = persist.tile([128, 1], BF, tag=f"bv{k}")
                nc.gpsimd.dma_start(out=t, in_=bv_d.ap()[k * 128:(k + 1) * 128, :])
                bv_sb.append(t)
                t = persist.tile([128, 1], F32, tag=f"bo{k}")
                nc.gpsimd.dma_start(out=t, in_=bo_d.ap()[k * 128:(k + 1) * 128, :])
                bo_sb.append(t)
                t = persist.tile([128, 1], F32, tag=f"gam{k}")
                nc.gpsimd.dma_start(out=t, in_=gam_d.ap()[k * 128:(k + 1) * 128, :])
                gam.append(t)
                t = persist.tile([128, 1], F32, tag=f"bet{k}")
                nc.gpsimd.dma_start(out=t, in_=bet_d.ap()[k * 128:(k + 1) * 128, :])
                bet.append(t)
            # q/k bias replicated across all 128 partitions (spatial rows)
            import concourse.bass as bass
            bqk_rep = persist.tile([128, 2 * C], BF, tag="bqk_rep")
            _bqk_ap = bqk_d.ap()
            nc.gpsimd.dma_start(
                out=bqk_rep,
                in_=bass.AP(tensor=_bqk_ap.tensor, offset=_bqk_ap.offset,
                            ap=[[0, 128], [1, 2 * C]]))

            zero1 = persist.tile([1, 128], BF, tag="zero1")
            nc.gpsimd.memset(zero1, 0.0)
            zrhs256 = persist.tile([1, 256], BF, tag="zrhs256")
            nc.gpsimd.memset(zrhs256, 0.0)
            ident = persist.tile([128, 128], BF, tag="ident")
            make_identity(nc, ident)
            eps_t = persist.tile([128, 1], F32, tag="eps")
            nc.gpsimd.memset(eps_t, EPS)
            # group indicator matrices (groupnorm cross-partition reduce)
            indf = []
            for k in range(CK):
                t = persist.tile([128, G], F32, tag=f"indf{k}")
                nc.gpsimd.dma_start(
                    out=t, in_=indf_d.ap()[k * 128:(k + 1) * 128, :])
                indf.append(t)
            indb = persist.tile([G, C], F32, tag="indb")
            nc.gpsimd.dma_start(out=indb, in_=indb_d.ap())

            # ---- per-batch phases (emitted software-pipelined below) ----
            def phase_norm(b):
                # x load (split DMAs so bn_stats can start on early columns)
                xs = []
                for k in range(CK):
                    t = big.tile([128, N], F32, tag=f"x{k}")
                    xq = min(1024, N)
                    for q4 in range(0, N, xq):
                        nc.sync.dma_start(
                            out=t[:, q4:q4 + xq],
                            in_=x_d.ap()[b, k * 128:(k + 1) * 128,
                                         q4:q4 + xq])
                    xs.append(t)

                # groupnorm stats: per-partition mean/var via bn_stats
                mvs = []
                for k in range(CK):
                    st = small.tile([128, SUB, 6], F32, tag=f"st{k}")
                    for j in range(SUB):
                        nc.vector.bn_stats(
                            out=st[:, j, :], in_=xs[k][:, j * 512:(j + 1) * 512])
                    mv = small.tile([128, 2], F32, tag=f"mv{k}")
                    nc.vector.bn_aggr(out=mv, in_=st)
                    mvs.append(mv)
                # rhs2: col0 = mean_p, col1 = mean_p^2 + var_p = E[x^2]_p
                rhs2s = []
                for k in range(CK):
                    r2 = small.tile([128, 2], F32, tag=f"r2{k}")
                    nc.gpsimd.tensor_copy(out=r2[:, 0:1], in_=mvs[k][:, 0:1])
                    nc.vector.scalar_tensor_tensor(
                        out=r2[:, 1:2], in0=mvs[k][:, 0:1],
                        scalar=mvs[k][:, 0:1], in1=mvs[k][:, 1:2],
                        op0=OP.mult, op1=OP.add)
                    rhs2s.append(r2)
                # cross-partition reduce to per-group stats [8, 2]
                pg = ps_big.tile([G, 2], F32, tag="pbig")
                for k in range(CK):
                    nc.tensor.matmul(pg, indf[k], rhs2s[k],
                                     start=(k == 0), stop=(k == CK - 1))
                sg = small.tile([G, 2], F32, tag="sg")
                nc.vector.tensor_copy(out=sg, in_=pg)
                t2 = small.tile([G, 1], F32, tag="t2")
                nc.vector.tensor_mul(out=t2, in0=sg[:, 0:1], in1=sg[:, 0:1])
                vs = small.tile([G, 1], F32, tag="vs")
                nc.vector.tensor_sub(out=vs, in0=sg[:, 1:2], in1=t2)
                # rstd = exp(-0.5 * ln(var + eps)); Ln/Exp share a table set
                lnv = small.tile([G, 1], F32, tag="lnv")
                nc.scalar.activation(out=lnv, in_=vs, func=AF.Ln,
                                     bias=eps_t[0:G, :], scale=1.0)
                rstd = small.tile([G, 1], F32, tag="rstd")
                nc.scalar.activation(out=rstd, in_=lnv, func=AF.Exp, scale=-0.5)
                bcr = small.tile([G, 2], F32, tag="bcr")
                nc.gpsimd.tensor_copy(out=bcr[:, 0:1], in_=sg[:, 0:1])
                nc.gpsimd.tensor_copy(out=bcr[:, 1:2], in_=rstd)
                # broadcast group stats back to channels; affine coeffs
                scs = []
                nbs = []
                for k in range(CK):
                    pbc = ps_big.tile([128, 2], F32, tag="pbig")
                    nc.tensor.matmul(pbc, indb[:, k * 128:(k + 1) * 128], bcr,
                                     start=True, stop=True)
                    sc = small.tile([128, 1], F32, tag=f"sc{k}")
                    nc.vector.tensor_mul(out=sc, in0=pbc[:, 1:2], in1=gam[k])
                    t4 = small.tile([128, 1], F32, tag=f"t4{k}")
                    nc.vector.tensor_scalar_mul(out=t4, in0=pbc[:, 0:1],
                                                scalar1=sc)
                    nb = small.tile([128, 1], F32, tag=f"nb{k}")
                    nc.vector.tensor_sub(out=nb, in0=bet[k], in1=t4)
                    scs.append(sc)
                    nbs.append(nb)

                # normalize: h = x * scale_c + bias_c  (bf16).
                # Column-major loop order: the first qk matmul needs the
                # first 128 columns of ALL FOUR chunks, so producing columns
                # across chunks first lets the consumer start ~9us earlier
                # than chunk-major order would.
                hs = []
                for k in range(CK):
                    hs.append(big.tile([128, N], BF, tag=f"h{k}",
                                       name=f"h{k}"))
                for t in range(NT):
                    sl = slice(t * 512, (t + 1) * 512)
                    for k in range(CK):
                        nc.vector.tensor_scalar(
                            out=hs[k][:, sl], in0=xs[k][:, sl],
                            scalar1=scs[k], scalar2=nbs[k],
                            op0=OP.mult, op1=OP.add)
                return hs

            def phase_qkv_setup(b):
                # scores accumulators: both packed tiles share one psum bank
                Tsc = ps_sc.tile([128, 256], F32, tag="sc01")
                T0 = Tsc[:, 0:128]
                T1 = Tsc[:, 128:256]
                # one full-width zeroing matmul: marks the bank's pending-zero
                # bits and writes 0 everywhere; every scores matmul overlaps
                # its AP, so ordering is guaranteed, and all quadrant matmuls
                # can then accumulate in any order.
                nc.tensor.matmul(Tsc, zero1, zrhs256, start=True, stop=False,
                                 skip_group_check=True)
                vsb = []
                for k in range(CK):
                    vsb.append(big.tile([128, N], BF, tag=f"v{k}",
                                        name=f"v{k}"))
                return T0, T1, vsb

            def qk_chunk(b, hs, s):
                # qk projection for one 128-row spatial chunk
                qk = mid.tile([128, 2 * C], BF, tag="qk", bufs=4)
                for half in range(2):
                    # one-bank psum tiles (3 rotating slots) so the next
                    # chunk's matmuls never wait on this chunk's evac
                    pqk = ps_qk.tile([128, 512], F32, tag="pqk")
                    wseg = slice(half * 512, (half + 1) * 512)
                    for k in range(CK):
                        nc.tensor.matmul(
                            pqk, hs[k][:, s * 128:(s + 1) * 128],
                            wqk[k][:, wseg], start=(k == 0),
                            stop=(k == CK - 1))
                    nc.scalar.copy(out=qk[:, wseg], in_=pqk)
                # q/k bias add (bf16 tensor_tensor runs in DVE 2x mode)
                nc.vector.tensor_add(out=qk, in0=qk, in1=bqk_rep)
                return qk

            def emit_scores(qk, T0, T1):
                for h in range(NH):
                    tt, l = divmod(h, 4)
                    T = T0 if tt == 0 else T1
                    pr, cs = _SCORE_SLOT[l]
                    nc.tensor.matmul(
                        T[pr:pr + 64, cs:cs + 64],
                        qk[:, h * 64:(h + 1) * 64],
                        qk[:, 512 + h * 64:512 + (h + 1) * 64],
                        start=False, stop=False, skip_group_check=True,
                        tile_position=(0, pr))

            def phase_qkv_run(b, hs, T0, T1, vsb, s0, s1):
                # qk + scores, with the v projection interleaved (one 512-col
                # block per 4 spatial chunks) so h slices are fully consumed
                # — and released for the next batch's normalize — as the
                # loop advances.
                for s in range(s0, s1):
                    qk = qk_chunk(b, hs, s)
                    emit_scores(qk, T0, T1)
                    if s % 4 == 3:
                        t = s // 4
                        hsl = slice(t * 512, (t + 1) * 512)
                        for oc in range(CK):
                            pv = ps_big.tile([128, 512], F32, tag="pbig")
                            for k in range(CK):
                                nc.tensor.matmul(
                                    pv, wv[k][:, oc * 128:(oc + 1) * 128],
                                    hs[k][:, hsl], start=(k == 0),
                                    stop=(k == CK - 1))
                            # tensor_scalar has a 2x-mode uop (CAST is 1x)
                            nc.vector.tensor_scalar_mul(
                                out=vsb[oc][:, hsl], in0=pv, scalar1=1.0)

            def phase_att_out(b, T0, T1, vsb):
                # softmax + transpose -> attT (bf16)
                # softmax without max-subtraction: logits = S/8 are bounded
                # well inside fp32 exp range for this distribution.
                attTs = []
                for tt, T in enumerate([T0, T1]):
                    p_f = small.tile([128, 128], F32, tag=f"p{tt}")
                    att_bf = small.tile([128, 128], BF, tag=f"abf{tt}")
                    nc.scalar.activation(out=p_f, in_=T, func=AF.Exp,
                                         scale=scale)
                    rsum = small.tile([128, 2], F32, tag=f"rsum{tt}")
                    nc.vector.reduce_sum(
                        out=rsum,
                        in_=p_f.rearrange("p (h e) -> p h e", h=2),
                        axis=AX.X)
                    rinv = small.tile([128, 2], F32, tag=f"rinv{tt}")
                    nc.vector.reciprocal(out=rinv, in_=rsum)
                    for half in range(2):
                        sl = slice(half * 64, (half + 1) * 64)
                        nc.vector.tensor_scalar_mul(
                            out=att_bf[:, sl], in0=p_f[:, sl],
                            scalar1=rinv[:, half:half + 1])
                    ptr = ps_big.tile([128, 128], BF, tag="pbig")
                    nc.tensor.transpose(ptr, att_bf, ident)
                    aT = small.tile([128, 128], BF, tag=f"aT{tt}")
                    nc.vector.tensor_copy(out=aT, in_=ptr)
                    attTs.append(aT)

                # c = att @ b_v per head -> [C, 1] fp32; folded into the hv
                # evacuation as a per-partition bias (hv' = hv + c), which
                # makes w_out @ hv' carry the whole v-bias term so the output
                # only needs + b_out + x afterwards.
                csb = []
                for k in range(CK):
                    pcv = ps_big.tile([128, 1], F32, tag="pbig")
                    aT = attTs[k // 2]
                    epr, ecs = _EVEN_SLOT[k % 2]
                    opr, ocs = _ODD_SLOT[k % 2]
                    nc.tensor.matmul(
                        pcv[0:64, :], aT[epr:epr + 64, ecs:ecs + 64],
                        bv_sb[k][0:64, :], start=True, stop=True,
                        tile_position=(0, 0), skip_group_check=True)
                    nc.tensor.matmul(
                        pcv[64:128, :], aT[opr:opr + 64, ocs:ocs + 64],
                        bv_sb[k][64:128, :], start=True, stop=True,
                        tile_position=(64, 64), skip_group_check=True)
                    ct = small.tile([128, 1], F32, tag=f"c{k}")
                    nc.vector.tensor_copy(out=ct, in_=pcv)
                    csb.append(ct)

                # hv = att @ v, out = w_out @ hv + btot + x
                for t in range(NT):
                    hsl = slice(t * 512, (t + 1) * 512)
                    hvs = []
                    for k in range(CK):
                        phv = ps_big.tile([128, 512], F32, tag="pbig")
                        aT = attTs[k // 2]
                        epr, ecs = _EVEN_SLOT[k % 2]
                        opr, ocs = _ODD_SLOT[k % 2]
                        nc.tensor.matmul(
                            phv[0:64, :], aT[epr:epr + 64, ecs:ecs + 64],
                            vsb[k][0:64, hsl], start=True, stop=True,
                            tile_position=(0, 0), skip_group_check=True)
                        nc.tensor.matmul(
                            phv[64:128, :], aT[opr:opr + 64, ocs:ocs + 64],
                            vsb[k][64:128, hsl], start=True, stop=True,
                            tile_position=(64, 64), skip_group_check=True)
                        hv = mid.tile([128, 512], BF, tag=f"hv{k}", bufs=2)
                        # evacuate + add the folded v-bias (DVE 2x mode)
                        nc.vector.tensor_scalar_add(out=hv, in0=phv,
                                                    scalar1=csb[k])
                        hvs.append(hv)
                    for oc in range(CK):
                        # out-psum gets its own 2-slot tag so it never waits
                        # on hv-psum recycling (and vice versa)
                        po = ps_big.tile([128, 512], F32, tag="pout")
                        for k in range(CK):
                            nc.tensor.matmul(
                                po, wo[k][:, oc * 128:(oc + 1) * 128], hvs[k],
                                start=(k == 0), stop=(k == CK - 1))
                        xr = mid.tile([128, 512], F32, tag="xr")
                        nc.sync.dma_start(
                            out=xr,
                            in_=x_d.ap()[b, oc * 128:(oc + 1) * 128, hsl])
                        fin = mid.tile([128, 512], F32, tag="fin")
                        nc.vector.scalar_tensor_tensor(
                            out=fin, in0=po, scalar=bo_sb[oc], in1=xr,
                            op0=OP.add, op1=OP.add)
                        # non-final batches store via the idle gpsimd queue so
                        # they never delay the next batch's x loads on the
                        # sync queue; the last batch stores via sync (HWDGE)
                        # to shorten the kernel-tail drain
                        dma_eng = nc.gpsimd if b < B - 1 else nc.sync
                        dma_eng.dma_start(
                            out=out_d.ap()[b, oc * 128:(oc + 1) * 128, hsl],
                            in_=fin)

            # software-pipelined emission: batch b+1's stats/normalize AND
            # its first PRE qk-projection chunks (scores deferred to avoid
            # an in-order queue cycle) are emitted ahead of batch b's
            # softmax/hv/out, so the tensor engine has filler work while
            # batch b's softmax chain runs on DVE/ACT.
            PRE = min(3, SP)
            hs_b = phase_norm(0)
            st_b = phase_qkv_setup(0)
            phase_qkv_run(0, hs_b, *st_b, 0, SP)
            for b in range(1, B):
                hs_n = phase_norm(b)
                stash = [qk_chunk(b, hs_n, s) for s in range(PRE)]
                phase_att_out(b - 1, *st_b)
                st_b = phase_qkv_setup(b)
                for qk in stash:
                    emit_scores(qk, st_b[0], st_b[1])
                phase_qkv_run(b, hs_n, *st_b, PRE, SP)
                hs_b = hs_n
            phase_att_out(B - 1, *st_b)

    nc.compile()
    return nc


def make_indicators():
    """Host-built groupnorm reduce/broadcast indicator matrices."""
    ch = np.arange(C)
    grp = ch // (C // G)
    indf = np.zeros((C, G), np.float32)
    indf[ch, grp] = 1.0 / (C // G)
    indb = np.zeros((G, C), np.float32)
    indb[grp, ch] = 1.0
    return indf, indb


_PROGRAM = None


def _get_program():
    global _PROGRAM
    if _PROGRAM is None:
        _PROGRAM = build_program()
    return _PROGRAM


def kernel(x, gamma, beta, w_qkv, b_qkv, w_out, b_out):
    x = np.asarray(x)
    B, C_, H, W = x.shape
    N = H * W
    assert C_ == C and B == 16 and N == 4096
    nc = _get_program()

    bf = ml_dtypes.bfloat16
    w_qkv = np.asarray(w_qkv, dtype=np.float32)
    wqkT = np.ascontiguousarray(w_qkv[:2 * C].T).astype(bf)
    wvT = np.ascontiguousarray(w_qkv[2 * C:].T).astype(bf)
    woT = np.ascontiguousarray(np.asarray(w_out, dtype=np.float32).T).astype(bf)
    b_qkv = np.asarray(b_qkv, dtype=np.float32)
    bqk = np.ascontiguousarray(b_qkv[:2 * C].reshape(1, -1)).astype(bf)
    bv = np.ascontiguousarray(b_qkv[2 * C:].reshape(-1, 1)).astype(bf)
    bo = np.ascontiguousarray(np.asarray(b_out, np.float32).reshape(-1, 1))
    gam = np.ascontiguousarray(np.asarray(gamma, np.float32).reshape(-1, 1))
    bet = np.ascontiguousarray(np.asarray(beta, np.float32).reshape(-1, 1))
    xr = np.ascontiguousarray(x.reshape(B, C, N).astype(np.float32))

    indf, indb = make_indicators()
    bpc = B // N_CORES
    in_maps = []
    for c in range(N_CORES):
        in_maps.append({
            "x": xr[c * bpc:(c + 1) * bpc],
            "wqkT": wqkT, "wvT": wvT, "woT": woT,
            "bqk": bqk, "bv": bv, "bo": bo,
            "gamma": gam, "beta": bet,
            "indf": indf, "indb": indb,
        })
    res = run_bass_kernel_spmd(nc, in_maps, core_ids=list(range(N_CORES)))
    out = np.concatenate([res.results[c]["out"] for c in range(N_CORES)],
                         axis=0)
    return out.reshape(B, C_, H, W).astype(np.float32)



# revision 2
# speedup vs baseline: 1.4703x; 1.4703x over previous
"""Trainium2 Bass kernel for nn_AttentionBlock (B=16, C=512, H=W=64, 8 heads).

Gram-matrix formulation, data-parallel over batch (2 batches/core, 8 cores).
Per batch:

  x_res = bf16(x)                         (residual + matmul operand)
  group stats from bn_stats(x_res) -> per-channel affine sc, nb
  folded weights:  Wq' = Wq diag(sc), Wk' = Wk diag(sc)   (per batch)
  xT per chunk via one XBAR dma_start_transpose           (DMA, ~5us each)
  G = x x^T  upper-triangle chunk blocks + PE-transpose mirrors
  U = G Wk'^T ; scores_h = Wq'_h U_h + rank-1 bias terms
  softmax -> att (block-diag head pairs)
  R_p = A_p^T Wo_p^T ; MT = sum_p Wv_p^T R_p ; MT' = diag(sc) MT
  out = MT'^T x_res + (M nb + Wo (A bv) + b_out) + x_res

The Gram trick + GroupNorm weight-folding removes the q/k/v projections
and att@v entirely: PE work drops from ~300us to ~146us per core vs the
direct formulation; softmax/stats ride on ACT/DVE under the PE stream,
transposes ride on the DMA xbar.
"""

import numpy as np
import ml_dtypes

import concourse.bacc as bacc
import concourse.tile as tile
from concourse import mybir
from concourse.bass_utils import run_bass_kernel_spmd
from concourse.masks import make_identity

BF = mybir.dt.bfloat16
F32 = mybir.dt.float32
AX = mybir.AxisListType
OP = mybir.AluOpType
AF = mybir.ActivationFunctionType

C = 512
N = 4096
NH = 8
D = 64
G = 8
CK = 4        # channel chunks of 128
SP = 32       # spatial chunks of 128
SUB = 8       # bn_stats subgroups (free dim <= 512)
EPS = 1e-5
N_CORES = 8
B = 2         # batches per core

# scores quadrant placement: head h -> tile tt=h//4, slot l=h%4
_SCORE_SLOT = {0: (0, 0), 1: (64, 64), 2: (64, 0), 3: (0, 64)}


def build_program(debug=False):
    scale = float(1.0 / np.sqrt(D))
    nc = bacc.Bacc("TRN2", target_bir_lowering=False, debug=debug,
                   num_devices=N_CORES)

    x_d = nc.dram_tensor("x", [B, C, N], F32, kind="ExternalInput")
    wqkT_d = nc.dram_tensor("wqkT", [C, 2 * C], BF, kind="ExternalInput")
    wvn_d = nc.dram_tensor("wv_nt", [C, C], BF, kind="ExternalInput")
    woT_d = nc.dram_tensor("woT", [C, C], BF, kind="ExternalInput")
    bca_d = nc.dram_tensor("bca", [2, C], F32, kind="ExternalInput")
    bcb_d = nc.dram_tensor("bcb", [2, C], F32, kind="ExternalInput")
    bv_d = nc.dram_tensor("bv", [C, 1], BF, kind="ExternalInput")
    bo_d = nc.dram_tensor("bo", [C, 1], F32, kind="ExternalInput")
    gam_d = nc.dram_tensor("gamma", [C, 1], F32, kind="ExternalInput")
    bet_d = nc.dram_tensor("beta", [C, 1], F32, kind="ExternalInput")
    indf_d = nc.dram_tensor("indf", [C, G], F32, kind="ExternalInput")
    indb_d = nc.dram_tensor("indb", [G, C], F32, kind="ExternalInput")
    out_d = nc.dram_tensor("out", [B, C, N], F32, kind="ExternalOutput")

    with tile.TileContext(nc) as tc:
        import contextlib
        ctx = contextlib.ExitStack()
        with ctx:
            persist = ctx.enter_context(tc.tile_pool(name="persist", bufs=1))
            stage = ctx.enter_context(tc.tile_pool(name="stage", bufs=4))
            xres = ctx.enter_context(tc.tile_pool(name="xres", bufs=2))
            xtp = ctx.enter_context(tc.tile_pool(name="xtp", bufs=1))
            wfp = ctx.enter_context(tc.tile_pool(name="wfp", bufs=2))
            gup = ctx.enter_context(tc.tile_pool(name="gup", bufs=1))
            rmt = ctx.enter_context(tc.tile_pool(name="rmt", bufs=2))
            small = ctx.enter_context(tc.tile_pool(name="small", bufs=2))
            fin = ctx.enter_context(tc.tile_pool(name="fin", bufs=3))
            # PSUM: every tile pads to a full bank; 8 banks total.
            # gps0/gps1 (2) + mm (2) + sc01 (1) + sm (1) + po (2) = 8.
            ps_g = ctx.enter_context(
                tc.tile_pool(name="ps_g", bufs=1, space="PSUM"))
            ps_mm = ctx.enter_context(
                tc.tile_pool(name="ps_mm", bufs=2, space="PSUM"))
            ps_sc = ctx.enter_context(
                tc.tile_pool(name="ps_sc", bufs=1, space="PSUM"))
            ps_sm = ctx.enter_context(
                tc.tile_pool(name="ps_sm", bufs=1, space="PSUM"))
            ps_po = ctx.enter_context(
                tc.tile_pool(name="ps_po", bufs=2, space="PSUM"))

            # ---- persistent weights / constants ----
            wqk = []
            bo_sb = []
            gam = []
            bet = []
            indf = []
            for k in range(CK):
                t = persist.tile([128, 2 * C], BF, tag=f"wqk{k}")
                nc.gpsimd.dma_start(out=t, in_=wqkT_d.ap()[k * 128:(k + 1) * 128, :])
                wqk.append(t)
                t = persist.tile([128, 1], F32, tag=f"bo{k}")
                nc.gpsimd.dma_start(out=t, in_=bo_d.ap()[k * 128:(k + 1) * 128, :])
                bo_sb.append(t)
                t = persist.tile([128, 1], F32, tag=f"gam{k}")
                nc.gpsimd.dma_start(out=t, in_=gam_d.ap()[k * 128:(k + 1) * 128, :])
                gam.append(t)
                t = persist.tile([128, 1], F32, tag=f"bet{k}")
                nc.gpsimd.dma_start(out=t, in_=bet_d.ap()[k * 128:(k + 1) * 128, :])
                bet.append(t)
                t = persist.tile([128, G], F32, tag=f"indf{k}")
                nc.gpsimd.dma_start(out=t, in_=indf_d.ap()[k * 128:(k + 1) * 128, :])
                indf.append(t)
            wvp = []
            wop = []
            bvp = []
            for p in range(CK):
                t = persist.tile([128, C], BF, tag=f"wvp{p}")
                nc.gpsimd.dma_start(out=t, in_=wvn_d.ap()[p * 128:(p + 1) * 128, :])
                wvp.append(t)
                t = persist.tile([128, C], BF, tag=f"wop{p}")
                nc.gpsimd.dma_start(out=t, in_=woT_d.ap()[p * 128:(p + 1) * 128, :])
                wop.append(t)
                t = persist.tile([128, 1], BF, tag=f"bvp{p}")
                nc.gpsimd.dma_start(out=t, in_=bv_d.ap()[p * 128:(p + 1) * 128, :])
                bvp.append(t)
            indb = persist.tile([G, C], F32, tag="indb")
            nc.gpsimd.dma_start(out=indb, in_=indb_d.ap())
            bca = persist.tile([2, C], F32, tag="bca")
            nc.gpsimd.dma_start(out=bca, in_=bca_d.ap())
            bcb = persist.tile([2, C], F32, tag="bcb")
            nc.gpsimd.dma_start(out=bcb, in_=bcb_d.ap())
            eps_t = persist.tile([128, 1], F32, tag="eps")
            nc.gpsimd.memset(eps_t, EPS)
            zero1 = persist.tile([1, 128], BF, tag="zero1")
            nc.gpsimd.memset(zero1, 0.0)
            zrhs256 = persist.tile([1, 256], BF, tag="zrhs256")
            nc.gpsimd.memset(zrhs256, 0.0)
            # block-diag att tiles: off-diag quadrants stay zero forever
            att_bd = []
            for p in range(CK):
                t = persist.tile([128, 128], BF, tag=f"attbd{p}")
                nc.gpsimd.memset(t, 0.0)
                att_bd.append(t)
            ident = persist.tile([128, 128], BF, tag="ident")
            make_identity(nc, ident)

            def emit_load_stats(b):
                # chunk-major load; one big XBAR transpose per chunk
                # (per-call overhead ~1.2us makes small calls prohibitive)
                xrs = [xres.tile([128, N], BF, tag=f"xr{k}", name=f"xr{k}_{b}")
                       for k in range(CK)]
                sts = [small.tile([128, SUB, 6], F32, tag=f"st{k}",
                                  name=f"st{k}_{b}")
                       for k in range(CK)]
                for k in range(CK):
                    for q in range(4):
                        csl = slice(q * 1024, (q + 1) * 1024)
                        stg = stage.tile([128, 1024], F32, tag="stg")
                        nc.sync.dma_start(
                            out=stg, in_=x_d.ap()[b, k * 128:(k + 1) * 128, csl])
                        nc.scalar.copy(out=xrs[k][:, csl], in_=stg)
                        for j2 in range(2):
                            j = q * 2 + j2
                            nc.vector.bn_stats(
                                out=sts[k][:, j, :],
                                in_=xrs[k][:, j * 512:(j + 1) * 512])
                # transposes after all load triggers (XBAR call blocks the
                # sync queue ~5us each); xtk[k][p, s, j] = x_res[k][j, s*128+p]
                xtk = []
                for k in range(CK):
                    t = xtp.tile([128, SP, 128], BF, tag=f"xtk{k}",
                                 name=f"xtk{k}_{b}")
                    nc.sync.dma_start_transpose(out=t, in_=xrs[k])
                    xtk.append(t)
                return xrs, sts, xtk

            def emit_stats_finish(b, sts):
                mvs = []
                rhs2s = []
                for k in range(CK):
                    mv = small.tile([128, 2], F32, tag=f"mv{k}")
                    nc.vector.bn_aggr(out=mv, in_=sts[k])
                    mvs.append(mv)
                    r2 = small.tile([128, 2], F32, tag=f"r2{k}")
                    nc.gpsimd.tensor_copy(out=r2[:, 0:1], in_=mv[:, 0:1])
                    nc.vector.scalar_tensor_tensor(
                        out=r2[:, 1:2], in0=mv[:, 0:1], scalar=mv[:, 0:1],
                        in1=mv[:, 1:2], op0=OP.mult, op1=OP.add)
                    rhs2s.append(r2)
                pg = ps_sm.tile([G, 2], F32, tag="sm", name=f"pg_{b}")
                for k in range(CK):
                    nc.tensor.matmul(pg, indf[k], rhs2s[k],
                                     start=(k == 0), stop=(k == CK - 1))
                sg = small.tile([G, 2], F32, tag="sg")
                nc.vector.tensor_copy(out=sg, in_=pg)
                t2 = small.tile([G, 1], F32, tag="t2")
                nc.vector.tensor_mul(out=t2, in0=sg[:, 0:1], in1=sg[:, 0:1])
                vs = small.tile([G, 1], F32, tag="vs")
                nc.vector.tensor_sub(out=vs, in0=sg[:, 1:2], in1=t2)
                lnv = small.tile([G, 1], F32, tag="lnv")
                nc.scalar.activation(out=lnv, in_=vs, func=AF.Ln,
                                     bias=eps_t[0:G, :], scale=1.0)
                rstd = small.tile([G, 1], F32, tag="rstd")
                nc.scalar.activation(out=rstd, in_=lnv, func=AF.Exp, scale=-0.5)
                bcr = small.tile([G, 2], F32, tag="bcr")
                nc.gpsimd.tensor_copy(out=bcr[:, 0:1], in_=sg[:, 0:1])
                nc.gpsimd.tensor_copy(out=bcr[:, 1:2], in_=rstd)
                scs = []
                ms = []
                stks = []
                wfs = []
                for k in range(CK):
                    pbc = ps_sm.tile([128, 2], F32, tag="sm", name=f"pbc{k}_{b}")
                    nc.tensor.matmul(pbc, indb[:, k * 128:(k + 1) * 128], bcr,
                                     start=True, stop=True)
                    sc = small.tile([128, 1], F32, tag=f"sc{k}",
                                    name=f"sc{k}_{b}")
                    nc.vector.tensor_mul(out=sc, in0=pbc[:, 1:2], in1=gam[k])
                    t4 = small.tile([128, 1], F32, tag=f"t4{k}")
                    nc.vector.tensor_scalar_mul(out=t4, in0=pbc[:, 0:1],
                                                scalar1=sc)
                    nb = small.tile([128, 1], F32, tag=f"nb{k}")
                    nc.vector.tensor_sub(out=nb, in0=bet[k], in1=t4)
                    rcp = small.tile([128, 1], F32, tag=f"rcp{k}")
                    nc.vector.reciprocal(out=rcp, in_=sc)
                    m = small.tile([128, 1], BF, tag=f"m{k}", name=f"m{k}_{b}")
                    nc.vector.tensor_mul(out=m, in0=nb, in1=rcp)
                    scxs = small.tile([128, 1], F32, tag=f"scxs{k}")
                    nc.vector.tensor_scalar(
                        out=scxs, in0=mvs[k][:, 0:1], scalar1=sc,
                        scalar2=float(N), op0=OP.mult, op1=OP.mult)
                    # lhsT cols: [nb, sc*xsum, sc*xsum + N*nb, nb] so each
                    # bias-row pair comes out in final row order
                    stk = small.tile([128, 4], BF, tag=f"stk{k}")
                    nc.gpsimd.tensor_copy(out=stk[:, 0:1], in_=nb)
                    nc.gpsimd.tensor_copy(out=stk[:, 1:2], in_=scxs)
                    nc.vector.scalar_tensor_tensor(
                        out=stk[:, 2:3], in0=nb, scalar=float(N), in1=scxs,
                        op0=OP.mult, op1=OP.add)
                    nc.gpsimd.tensor_copy(out=stk[:, 3:4], in_=nb)
                    wf = wfp.tile([128, 2 * C], BF, tag=f"wf{k}",
                                  name=f"wf{k}_{b}")
                    nc.vector.tensor_scalar_mul(out=wf, in0=wqk[k], scalar1=sc)
                    scs.append(sc)
                    ms.append(m)
                    stks.append(stk)
                    wfs.append(wf)
                # bias rows (brq/brk share the single "sm" psum slot):
                # brq rows = [Wq.nb ; Wq.(sc*xsum)]        + bca=[bq; 0]
                # brk rows = [Wk.(sc*xsum + N*nb) ; Wk.nb] + bcb=[N*bk; bk]
                # -> v_l = [bq' ; Sq], v_r = [Sk + N*bk' ; bk']
                v_l = small.tile([2, C], BF, tag="vl", name=f"vl_{b}")
                v_r = small.tile([2, C], BF, tag="vr", name=f"vr_{b}")
                brq = ps_sm.tile([2, C], F32, tag="sm", name=f"brq_{b}")
                for k in range(CK):
                    nc.tensor.matmul(brq, stks[k][:, 0:2], wqk[k][:, 0:C],
                                     start=(k == 0), stop=(k == CK - 1))
                nc.vector.tensor_add(out=v_l, in0=brq, in1=bca)
                brk = ps_sm.tile([2, C], F32, tag="sm", name=f"brk_{b}")
                for k in range(CK):
                    nc.tensor.matmul(brk, stks[k][:, 2:4], wqk[k][:, C:2 * C],
                                     start=(k == 0), stop=(k == CK - 1))
                nc.vector.tensor_add(out=v_r, in0=brk, in1=bcb)
                return scs, ms, wfs, v_l, v_r

            def emit_gx(b, xtk):
                # upper-triangle chunk blocks (G symmetric), two psum banks
                # per row-pair half; block order follows transpose arrival
                gsb = [gup.tile([128, C], BF, tag=f"g{oc}", name=f"g{oc}_{b}")
                       for oc in range(CK)]
                halves = [
                    ((0, 1), [(0, 0), (0, 1), (1, 1), (0, 2), (1, 2),
                              (0, 3), (1, 3)]),
                    ((2, 3), [(2, 2), (2, 3), (3, 3)]),
                ]
                for rows, blocks in halves:
                    gps = {row: ps_g.tile([128, C], F32, tag=f"gps{i}",
                                          name=f"gps{row}_{b}")
                           for i, row in enumerate(rows)}
                    for ci, cj in blocks:
                        dst = gps[ci][:, cj * 128:(cj + 1) * 128]
                        for s in range(SP):
                            nc.tensor.matmul(
                                dst, xtk[ci][:, s, :], xtk[cj][:, s, :],
                                start=(s == 0), stop=(s == SP - 1))
                    for row in rows:
                        nc.scalar.copy(out=gsb[row][:, row * 128:],
                                       in_=gps[row][:, row * 128:])
                # mirror sub-diagonal blocks via PE transpose
                for ci in range(1, CK):
                    for cj in range(ci):
                        tp = ps_mm.tile([128, 128], BF, tag="mm",
                                        name=f"mir{ci}{cj}_{b}")
                        nc.tensor.transpose(
                            tp, gsb[cj][:, ci * 128:(ci + 1) * 128], ident)
                        nc.scalar.copy(
                            out=gsb[ci][:, cj * 128:(cj + 1) * 128], in_=tp)
                return gsb

            def emit_u_scores(b, gsb, wfs, v_l, v_r):
                usb = []
                for oc in range(CK):
                    ups = ps_mm.tile([128, C], F32, tag="mm",
                                     name=f"ups{oc}_{b}")
                    for c in range(CK):
                        nc.tensor.matmul(
                            ups, gsb[c][:, oc * 128:(oc + 1) * 128],
                            wfs[c][:, C:2 * C], start=(c == 0),
                            stop=(c == CK - 1))
                    t = gup.tile([128, C], BF, tag=f"u{oc}")
                    nc.scalar.copy(out=t, in_=ups)
                    usb.append(t)
                Tsc = ps_sc.tile([128, 256], F32, tag="sc01")
                nc.tensor.matmul(Tsc, zero1, zrhs256, start=True, stop=False,
                                 skip_group_check=True)
                for h in range(NH):
                    tt, l = divmod(h, 4)
                    pr, cs = _SCORE_SLOT[l]
                    T = Tsc[:, tt * 128:(tt + 1) * 128]
                    hsl = slice(h * D, (h + 1) * D)
                    for c in range(CK):
                        nc.tensor.matmul(
                            T[pr:pr + 64, cs:cs + 64], wfs[c][:, hsl],
                            usb[c][:, hsl], start=False, stop=False,
                            skip_group_check=True, tile_position=(0, pr))
                    nc.tensor.matmul(
                        T[pr:pr + 64, cs:cs + 64], v_l[:, hsl], v_r[:, hsl],
                        start=False, stop=False, skip_group_check=True,
                        tile_position=(0, pr))
                return Tsc

            def emit_softmax(b, Tsc):
                for tt in range(2):
                    T = Tsc[:, tt * 128:(tt + 1) * 128]
                    p_f = small.tile([128, 128], F32, tag=f"p{tt}")
                    nc.scalar.activation(out=p_f, in_=T, func=AF.Exp,
                                         scale=scale)
                    rsum = small.tile([128, 2], F32, tag=f"rsum{tt}")
                    nc.vector.reduce_sum(
                        out=rsum,
                        in_=p_f.rearrange("p (h e) -> p h e", h=2),
                        axis=AX.X)
                    rinv = small.tile([128, 2], F32, tag=f"rinv{tt}")
                    nc.vector.reciprocal(out=rinv, in_=rsum)
                    for l in range(4):
                        h = tt * 4 + l
                        pr, cs = _SCORE_SLOT[l]
                        q = h % 2
                        dst = att_bd[h // 2][q * 64:(q + 1) * 64,
                                             q * 64:(q + 1) * 64]
                        nc.vector.tensor_scalar_mul(
                            out=dst, in0=p_f[pr:pr + 64, cs:cs + 64],
                            scalar1=rinv[pr:pr + 64, cs // 64:cs // 64 + 1])

            def emit_r_mt(b, scs, ms):
                rsb = []
                for p in range(CK):
                    rps = ps_mm.tile([128, C], F32, tag="mm", name=f"rps{p}_{b}")
                    nc.tensor.matmul(rps, att_bd[p], wop[p],
                                     start=True, stop=True)
                    t = rmt.tile([128, C], BF, tag=f"r{p}")
                    nc.scalar.copy(out=t, in_=rps)
                    rsb.append(t)
                mtfs = []
                for c in range(CK):
                    mtps = ps_mm.tile([128, C], F32, tag="mm",
                                      name=f"mtps{c}_{b}")
                    for p in range(CK):
                        nc.tensor.matmul(
                            mtps, wvp[p][:, c * 128:(c + 1) * 128], rsb[p],
                            start=(p == 0), stop=(p == CK - 1))
                    t = rmt.tile([128, C], BF, tag=f"mtf{c}",
                                 name=f"mtf{c}_{b}")
                    nc.vector.tensor_scalar_mul(out=t, in0=mtps, scalar1=scs[c])
                    mtfs.append(t)
                # const vector: M.nb + Wo.(A.bv) per out-chunk
                cps = ps_sm.tile([128, CK], F32, tag="sm", name=f"cps_{b}")
                bconsts = []
                for oc in range(CK):
                    osl = slice(oc * 128, (oc + 1) * 128)
                    for c in range(CK):
                        nc.tensor.matmul(cps[:, oc:oc + 1], mtfs[c][:, osl],
                                         ms[c], start=(c == 0), stop=False)
                    for p in range(CK):
                        nc.tensor.matmul(cps[:, oc:oc + 1], rsb[p][:, osl],
                                         bvp[p], start=False,
                                         stop=(p == CK - 1))
                    t = small.tile([128, 1], F32, tag=f"bc{oc}",
                                   name=f"bc{oc}_{b}")
                    nc.vector.tensor_add(out=t, in0=cps[:, oc:oc + 1],
                                         in1=bo_sb[oc])
                    bconsts.append(t)
                return mtfs, bconsts

            def emit_out(b, xrs, mtfs, bconsts):
                for oc in range(CK):
                    osl = slice(oc * 128, (oc + 1) * 128)
                    for tp in range(4):
                        pos = [ps_po.tile([128, 512], F32, tag="po",
                                          name=f"po{j_}_{b}")
                               for j_ in range(2)]
                        for c in range(CK):
                            for j in range(2):
                                t = tp * 2 + j
                                nc.tensor.matmul(
                                    pos[j], mtfs[c][:, osl],
                                    xrs[c][:, t * 512:(t + 1) * 512],
                                    start=(c == 0), stop=(c == CK - 1))
                        for j in range(2):
                            t = tp * 2 + j
                            tsl = slice(t * 512, (t + 1) * 512)
                            f = fin.tile([128, 512], F32, tag="fin")
                            nc.vector.scalar_tensor_tensor(
                                out=f, in0=pos[j], scalar=bconsts[oc],
                                in1=xrs[oc][:, tsl], op0=OP.add, op1=OP.add)
                            dma_eng = nc.gpsimd if b < B - 1 else nc.sync
                            dma_eng.dma_start(
                                out=out_d.ap()[b, oc * 128:(oc + 1) * 128, tsl],
                                in_=f)

            # ---- pipelined emission over the 2 batches ----
            xrs0, sts0, xtk0 = emit_load_stats(0)
            st0 = emit_stats_finish(0, sts0)
            gsb0 = emit_gx(0, xtk0)
            xrs1, sts1, xtk1 = emit_load_stats(1)
            st1 = emit_stats_finish(1, sts1)
            Tsc0 = emit_u_scores(0, gsb0, st0[2], st0[3], st0[4])
            emit_softmax(0, Tsc0)
            gsb1 = emit_gx(1, xtk1)
            mtfs0, bc0 = emit_r_mt(0, st0[0], st0[1])
            emit_out(0, xrs0, mtfs0, bc0)
            Tsc1 = emit_u_scores(1, gsb1, st1[2], st1[3], st1[4])
            emit_softmax(1, Tsc1)
            mtfs1, bc1 = emit_r_mt(1, st1[0], st1[1])
            emit_out(1, xrs1, mtfs1, bc1)

    nc.compile()
    return nc


def make_indicators():
    ch = np.arange(C)
    grp = ch // (C // G)
    indf = np.zeros((C, G), np.float32)
    indf[ch, grp] = 1.0 / (C // G)
    indb = np.zeros((G, C), np.float32)
    indb[grp, ch] = 1.0
    return indf, indb


def prep_inputs(x, gamma, beta, w_qkv, b_qkv, w_out, b_out):
    bf = ml_dtypes.bfloat16
    w_qkv = np.asarray(w_qkv, dtype=np.float32)
    b_qkv = np.asarray(b_qkv, dtype=np.float32)
    wqkT = np.ascontiguousarray(w_qkv[:2 * C].T).astype(bf)
    wv_nt = np.ascontiguousarray(w_qkv[2 * C:]).astype(bf)
    woT = np.ascontiguousarray(np.asarray(w_out, np.float32).T).astype(bf)
    bq_h = b_qkv[:C]
    bk_h = b_qkv[C:2 * C]
    bca = np.ascontiguousarray(
        np.stack([bq_h, np.zeros_like(bq_h)])).astype(np.float32)
    bcb = np.ascontiguousarray(
        np.stack([float(N) * bk_h, bk_h])).astype(np.float32)
    bv = np.ascontiguousarray(b_qkv[2 * C:].reshape(C, 1)).astype(bf)
    bo = np.ascontiguousarray(np.asarray(b_out, np.float32).reshape(C, 1))
    gam = np.ascontiguousarray(np.asarray(gamma, np.float32).reshape(C, 1))
    bet = np.ascontiguousarray(np.asarray(beta, np.float32).reshape(C, 1))
    indf, indb = make_indicators()
    xr = np.ascontiguousarray(
        np.asarray(x, np.float32).reshape(16, C, N))
    base = {"wqkT": wqkT, "wv_nt": wv_nt, "woT": woT, "bca": bca,
            "bcb": bcb, "bv": bv, "bo": bo, "gamma": gam, "beta": bet,
            "indf": indf, "indb": indb}
    in_maps = []
    for c in range(N_CORES):
        d = dict(base)
        d["x"] = xr[c * B:(c + 1) * B]
        in_maps.append(d)
    return in_maps


_PROGRAM = None


def _get_program():
    global _PROGRAM
    if _PROGRAM is None:
        _PROGRAM = build_program()
    return _PROGRAM


def kernel(x, gamma, beta, w_qkv, b_qkv, w_out, b_out):
    x = np.asarray(x)
    Bt, C_, H, W = x.shape
    assert C_ == C and Bt == 16 and H * W == N
    nc = _get_program()
    in_maps = prep_inputs(x, gamma, beta, w_qkv, b_qkv, w_out, b_out)
    res = run_bass_kernel_spmd(nc, in_maps, core_ids=list(range(N_CORES)))
    out = np.concatenate([res.results[c]["out"] for c in range(N_CORES)],
                         axis=0)
    return out.reshape(Bt, C_, H, W).astype(np.float32)
